# revision 18
# baseline (speedup 1.0000x reference)
"""Trainium2 Bass kernel for nn_DiacriticRestorer (2-layer biLSTM encoder +
2-layer LSTM decoder + linear head), data-parallel over batch on 8 NeuronCores.

Layout conventions (per core, local batch BL=8):
 - All recurrent state and activations are "transposed": hidden dim on SBUF
   partitions (split into 128-row chunks), batch on the free dim.
   hT tile [128, NK*BL]: element [p, k*BL+b] = h[b, k*128+p].
 - Gates are computed as gatesT = Whh @ h (+ xg) with the weight chunk
   stationary: out chunk [128 gate rows, BL batch cols]. No transposes
   anywhere in the time loop.
 - Input projections xg (= x_t @ Wih.T + bias) are precomputed per stage as
   batched GEMMs over all T*BL tokens and streamed from DRAM during the scan.
 - Layer-0 input projections collapse to (one-hot tokens) @ (emb @ Wih.T + b):
   the [V=128, 4H] product table is computed on host; the one-hot GEMM runs on
   device (K = V = 128, a single contraction chunk).
 - All matmul operands fp16 (stationary fp16 enables fast weight load); PSUM
   accumulation, cell state, gate elementwise in fp32.
"""

import numpy as np

import concourse.bacc as bacc
import concourse.bass as bass
import concourse.mybir as mybir
import concourse.tile as tile
from concourse.bass import ds
from concourse.bass_utils import run_bass_kernel_spmd
from contextlib import ExitStack

F16 = mybir.dt.float16
F32 = mybir.dt.float32
AF = mybir.ActivationFunctionType
ALU = mybir.AluOpType

V, E, H, B = 128, 256, 512, 64
NCORES = 8
BL = B // NCORES  # 8
G = 4 * H  # 2048
NK = H // 128  # 4  h chunks
NM = G // 128  # 16 gate chunks
P = 128


def build_model(nc, T):
    TOK = BL * T
    NTT = min(512, TOK)  # tokens per GEMM tile
    NTILES = TOK // NTT
    TT = NTT // BL  # timesteps per GEMM tile

    ein = lambda name, shape, dt=F16: nc.dram_tensor(
        name, shape, dt, kind="ExternalInput"
    )

    onehotT = ein("onehotT", [P, TOK])
    tab_f = ein("tab_f", [P, G])
    tab_b = ein("tab_b", [P, G])
    tab_d = ein("tab_d", [P, G])
    whh = {
        k: ein(f"whhT_{k}", [P, NK * G])
        for k in ("l0f", "l0b", "l1f", "l1b", "d0", "d1")
    }
    wih_l1f = ein("wihT_l1f", [P, 8 * G])
    wih_l1b = ein("wihT_l1b", [P, 8 * G])
    wih_d0e = ein("wihT_d0e", [P, 8 * G])
    wih_d1 = ein("wihT_d1", [P, 4 * G])
    owT = ein("owT", [P, NK * V])
    bias_l1f = ein("bias_l1f", [P, NM], F32)
    bias_l1b = ein("bias_l1b", [P, NM], F32)
    bias_d1 = ein("bias_d1", [P, NM], F32)
    bias_out = ein("bias_out", [P, 1], F32)

    logitsT = nc.dram_tensor("logitsT", [P, TOK], F32, kind="ExternalOutput")

    # internal DRAM scratch
    xg = {
        k: nc.dram_tensor(f"xg_{k}", [P, T, NM * BL], F32)
        for k in ("af", "ab", "bf", "bb")
    }
    ybuf = {
        k: nc.dram_tensor(f"y_{k}", [P, T, NK * BL], F16)
        for k in ("l0f", "l0b", "l1f", "l1b", "d0", "d1")
    }

    with tile.TileContext(nc) as tc, ExitStack() as ctx:
        const = ctx.enter_context(tc.tile_pool(name="const", bufs=1))
        wpool = ctx.enter_context(tc.tile_pool(name="wpool", bufs=1))
        whpool = ctx.enter_context(tc.tile_pool(name="whpool", bufs=1))
        rhspool = ctx.enter_context(tc.tile_pool(name="rhs", bufs=12))
        pspool = ctx.enter_context(tc.tile_pool(name="ps", bufs=4, space="PSUM"))
        xsbpool = ctx.enter_context(tc.tile_pool(name="xsb", bufs=4))
        spool = ctx.enter_context(tc.tile_pool(name="scan", bufs=3))
        state = ctx.enter_context(tc.tile_pool(name="state", bufs=1))

        oh_sb = const.tile([P, TOK], F16)
        nc.sync.dma_start(oh_sb[:], onehotT[:])
        bias_sb = {}
        for nm, t in (("l1f", bias_l1f), ("l1b", bias_l1b), ("d1", bias_d1)):
            bias_sb[nm] = const.tile([P, NM], F32, name=f"bias_{nm}", tag=f"bias_{nm}")
            nc.sync.dma_start(bias_sb[nm][:], t[:])
        bout_sb = const.tile([P, 1], F32)
        nc.sync.dma_start(bout_sb[:], bias_out[:])

        hT = state.tile([P, NK * BL], F16)
        cT = state.tile([P, NK * BL], F32)
        sav_hf = state.tile([P, NK * BL], F16)
        sav_cf = state.tile([P, NK * BL], F32)
        sav_hb = state.tile([P, NK * BL], F16)
        sav_cb = state.tile([P, NK * BL], F32)

        def barrier():
            tc.strict_bb_all_engine_barrier()

        def onehot_gemm(tab_sb, xg_dram):
            """xg[t] = table[x[t]]  via one-hot GEMM. table includes bias."""
            for nt in range(NTILES):
                rhs = oh_sb[:, nt * NTT : (nt + 1) * NTT]
                for m in range(NM):
                    ps = pspool.tile([P, NTT], F32)
                    nc.tensor.matmul(
                        ps[:], tab_sb[:, m * P : (m + 1) * P], rhs, start=True, stop=True
                    )
                    xsb = xsbpool.tile([P, NTT], F32)
                    nc.scalar.activation(xsb[:], ps[:], AF.Identity)
                    # dram view [p, t(TT), b] at token tile nt, gate chunk m
                    dst = xg_dram[:, ds(nt * TT, TT), m * BL : (m + 1) * BL]
                    nc.sync.dma_start(dst, xsb[:].rearrange("p (t b) -> p t b", b=BL))

        def load_y_rhs(src_dram, nt, k):
            """rhs tile [128, NTT] fp16 = y chunk k, token tile nt."""
            rhs = rhspool.tile([P, NTT], F16)
            src = src_dram[:, ds(nt * TT, TT), k * BL : (k + 1) * BL]
            nc.sync.dma_start(rhs[:].rearrange("p (t b) -> p t b", b=BL), src)
            return rhs

        def proj_gemm(wih_sb, nkc, rhs_tiles_fn, bias_tile, xg_dram, extra_first=None):
            """xg = sum_k WihT[k-chunk] @ rhs_k (+ table one-hot term) + bias."""
            for nt in range(NTILES):
                rhs_tiles = rhs_tiles_fn(nt)
                for m in range(NM):
                    ps = pspool.tile([P, NTT], F32)
                    first = True
                    if extra_first is not None:
                        tab_sb = extra_first
                        nc.tensor.matmul(
                            ps[:],
                            tab_sb[:, m * P : (m + 1) * P],
                            oh_sb[:, nt * NTT : (nt + 1) * NTT],
                            start=True,
                            stop=False,
                        )
                        first = False
                    for k in range(nkc):
                        nc.tensor.matmul(
                            ps[:],
                            wih_sb[:, k * G + m * P : k * G + (m + 1) * P],
                            rhs_tiles[k][:],
                            start=first,
                            stop=(k == nkc - 1),
                        )
                        first = False
                    xsb = xsbpool.tile([P, NTT], F32)
                    if bias_tile is not None:
                        nc.scalar.activation(
                            xsb[:], ps[:], AF.Identity, bias=bias_tile[:, m : m + 1]
                        )
                    else:
                        nc.scalar.activation(xsb[:], ps[:], AF.Identity)
                    dst = xg_dram[:, ds(nt * TT, TT), m * BL : (m + 1) * BL]
                    nc.sync.dma_start(dst, xsb[:].rearrange("p (t b) -> p t b", b=BL))

        def scan(whh_in, xg_dram, y_dram, rev, init, save):
            whh_sb = whpool.tile([P, NK * G], F16, tag="whh")
            nc.sync.dma_start(whh_sb[:], whh_in[:])
            if init is None:
                nc.vector.memset(hT[:], 0.0)
                nc.vector.memset(cT[:], 0.0)
            else:
                nc.vector.tensor_copy(hT[:], init[0][:])
                nc.vector.tensor_copy(cT[:], init[1][:])
            barrier()
            with tc.For_i(0, T) as iv:
                t = (T - 1) - iv if rev else iv
                xg_sb = spool.tile([P, NM * BL], F32, tag="xg")
                nc.sync.dma_start(
                    xg_sb[:].rearrange("p (o c) -> p o c", o=1),
                    xg_dram[:, ds(t, 1), :],
                )
                ps = pspool.tile([P, NM * BL], F32, tag="scanps")
                for m in range(NM):
                    for k in range(NK):
                        nc.tensor.matmul(
                            ps[:, m * BL : (m + 1) * BL],
                            whh_sb[:, k * G + m * P : k * G + (m + 1) * P],
                            hT[:, k * BL : (k + 1) * BL],
                            start=(k == 0),
                            stop=(k == NK - 1),
                        )
                gat = spool.tile([P, NM * BL], F32, tag="gat")
                nc.vector.tensor_tensor(gat[:], ps[:], xg_sb[:], ALU.add)
                q = NK * BL  # 32 cols per gate
                act = spool.tile([P, NM * BL], F32, tag="act")
                nc.scalar.activation(act[:, 0 : q], gat[:, 0 : q], AF.Sigmoid)
                nc.scalar.activation(act[:, q : 2 * q], gat[:, q : 2 * q], AF.Sigmoid)
                nc.scalar.activation(act[:, 2 * q : 3 * q], gat[:, 2 * q : 3 * q], AF.Tanh)
                nc.scalar.activation(act[:, 3 * q : 4 * q], gat[:, 3 * q : 4 * q], AF.Sigmoid)
                ig = spool.tile([P, q], F32, tag="ig")
                nc.vector.tensor_tensor(ig[:], act[:, 0:q], act[:, 2 * q : 3 * q], ALU.mult)
                nc.vector.tensor_tensor(cT[:], act[:, q : 2 * q], cT[:], ALU.mult)
                nc.vector.tensor_tensor(cT[:], cT[:], ig[:], ALU.add)
                tc_t = spool.tile([P, q], F32, tag="tc")
                nc.scalar.activation(tc_t[:], cT[:], AF.Tanh)
                nc.vector.tensor_tensor(hT[:], act[:, 3 * q : 4 * q], tc_t[:], ALU.mult)
                nc.sync.dma_start(
                    y_dram[:, ds(t, 1), :],
                    hT[:].rearrange("p (o c) -> p o c", o=1),
                )
            if save is not None:
                nc.vector.tensor_copy(save[0][:], hT[:])
                nc.vector.tensor_copy(save[1][:], cT[:])
            barrier()

        # ---- phase 1: layer-0 input projections (table gathers) ----
        tabf_sb = wpool.tile([P, G], F16, tag="tab")
        nc.sync.dma_start(tabf_sb[:], tab_f[:])
        tabb_sb = wpool.tile([P, G], F16, tag="tab2")
        nc.sync.dma_start(tabb_sb[:], tab_b[:])
        onehot_gemm(tabf_sb, xg["af"])
        onehot_gemm(tabb_sb, xg["ab"])
        barrier()

        # ---- layer-0 scans ----
        scan(whh["l0f"], xg["af"], ybuf["l0f"], False, None, (sav_hf, sav_cf))
        scan(whh["l0b"], xg["ab"], ybuf["l0b"], True, None, (sav_hb, sav_cb))

        # ---- layer-1 input projections ----
        wf_sb = wpool.tile([P, 8 * G], F16, tag="wih")
        nc.sync.dma_start(wf_sb[:], wih_l1f[:])
        wb_sb = wpool.tile([P, 8 * G], F16, tag="wih2")
        nc.sync.dma_start(wb_sb[:], wih_l1b[:])

        def l1_rhs(nt):
            return [load_y_rhs(ybuf["l0f"], nt, k) for k in range(NK)] + [
                load_y_rhs(ybuf["l0b"], nt, k) for k in range(NK)
            ]

        for nt in range(NTILES):
            tiles = l1_rhs(nt)
            for wsb, bt, dst in ((wf_sb, bias_sb["l1f"], "bf"), (wb_sb, bias_sb["l1b"], "bb")):
                for m in range(NM):
                    ps = pspool.tile([P, NTT], F32)
                    for k in range(8):
                        nc.tensor.matmul(
                            ps[:],
                            wsb[:, k * G + m * P : k * G + (m + 1) * P],
                            tiles[k][:],
                            start=(k == 0),
                            stop=(k == 7),
                        )
                    xsb = xsbpool.tile([P, NTT], F32)
                    nc.scalar.activation(xsb[:], ps[:], AF.Identity, bias=bt[:, m : m + 1])
                    d = xg[dst][:, ds(nt * TT, TT), m * BL : (m + 1) * BL]
                    nc.sync.dma_start(d, xsb[:].rearrange("p (t b) -> p t b", b=BL))
        barrier()

        # ---- layer-1 scans ----
        scan(whh["l1f"], xg["bf"], ybuf["l1f"], False, None, None)
        scan(whh["l1b"], xg["bb"], ybuf["l1b"], True, None, None)

        # ---- decoder layer-0 input projection (emb table + enc_out GEMM) ----
        wd_sb = wpool.tile([P, 8 * G], F16, tag="wih")
        nc.sync.dma_start(wd_sb[:], wih_d0e[:])
        tabd_sb = wpool.tile([P, G], F16, tag="tab")
        nc.sync.dma_start(tabd_sb[:], tab_d[:])

        def d0_rhs(nt):
            return [load_y_rhs(ybuf["l1f"], nt, k) for k in range(NK)] + [
                load_y_rhs(ybuf["l1b"], nt, k) for k in range(NK)
            ]

        proj_gemm(wd_sb, 8, d0_rhs, None, xg["af"], extra_first=tabd_sb)
        barrier()

        # ---- decoder scans ----
        scan(whh["d0"], xg["af"], ybuf["d0"], False, (sav_hf, sav_cf), None)

        wd1_sb = wpool.tile([P, 4 * G], F16, tag="wih")
        nc.sync.dma_start(wd1_sb[:], wih_d1[:])

        def d1_rhs(nt):
            return [load_y_rhs(ybuf["d0"], nt, k) for k in range(NK)]

        proj_gemm(wd1_sb, NK, d1_rhs, bias_sb["d1"], xg["bf"])
        barrier()

        scan(whh["d1"], xg["bf"], ybuf["d1"], False, (sav_hb, sav_cb), None)

        # ---- output projection ----
        ow_sb = wpool.tile([P, NK * V], F16, tag="tab")
        nc.sync.dma_start(ow_sb[:], owT[:])
        for nt in range(NTILES):
            tiles = [load_y_rhs(ybuf["d1"], nt, k) for k in range(NK)]
            ps = pspool.tile([P, NTT], F32)
            for k in range(NK):
                nc.tensor.matmul(
                    ps[:],
                    ow_sb[:, k * V : (k + 1) * V],
                    tiles[k][:],
                    start=(k == 0),
                    stop=(k == NK - 1),
                )
            xsb = xsbpool.tile([P, NTT], F32)
            nc.scalar.activation(xsb[:], ps[:], AF.Identity, bias=bout_sb[:])
            nc.sync.dma_start(logitsT[:, nt * NTT : (nt + 1) * NTT], xsb[:])

    nc.finalize()
    return nc


# ---------------- host-side packing ----------------


def _pack_whhT(Whh):
    out = np.empty((P, NK * G), np.float16)
    for k in range(NK):
        out[:, k * G : (k + 1) * G] = Whh[:, k * P : (k + 1) * P].T
    return out


def _pack_wihT(Wih, col_off, nkc):
    out = np.empty((P, nkc * G), np.float16)
    for k in range(nkc):
        c = col_off + k * P
        out[:, k * G : (k + 1) * G] = Wih[:, c : c + P].T
    return out


def _pack_table(emb, Wih_sub, bias):
    tab = emb.astype(np.float64) @ Wih_sub.astype(np.float64).T + bias.astype(np.float64)
    return tab.astype(np.float16)  # [V, G]


def _pack_bias(bih, bhh):
    b = (bih + bhh).astype(np.float32)
    return b.reshape(NM, P).T.copy()  # [p, m]


_CACHE = {}
LAST_EXEC_NS = None


def _run_spmd_timed(nc, in_maps, iters=3):
    """Mirror run_bass_via_pjrt's multi-core path, but device_put inputs once
    so repeated executions time (exec + dispatch), not input upload."""
    import time as _time

    import jax
    import jax.numpy as jnp
    import concourse.mybir as mybir_
    from concourse import bass2jax
    from jax.experimental.shard_map import shard_map
    from jax.sharding import Mesh, NamedSharding, PartitionSpec

    bass2jax.install_neuronx_cc_hook()
    n_cores = len(in_maps)
    partition_name = nc.partition_id_tensor.name if nc.partition_id_tensor else None

    in_names, out_names, out_avals, zero_outs = [], [], [], []
    for alloc in nc.m.functions[0].allocations:
        if not isinstance(alloc, mybir_.MemoryLocationSet):
            continue
        name = alloc.memorylocations[0].name
        if alloc.kind == "ExternalInput":
            if name != partition_name:
                in_names.append(name)
        elif alloc.kind == "ExternalOutput":
            out_names.append(name)
            shape = tuple(alloc.tensor_shape)
            dtype = mybir_.dt.np(alloc.dtype)
            out_avals.append(jax.core.ShapedArray(shape, dtype))
            zero_outs.append(np.zeros(shape, dtype))
    n_params = len(in_names)
    n_outs = len(out_avals)
    all_in_names = list(in_names) + list(out_names)
    if partition_name is not None:
        all_in_names.append(partition_name)

    donate = tuple(range(n_params, n_params + n_outs))

    def _body(*args):
        operands = list(args)
        if partition_name is not None:
            operands.append(bass2jax.partition_id_tensor())
        outs = bass2jax._bass_exec_p.bind(
            *operands,
            out_avals=tuple(out_avals),
            in_names=tuple(all_in_names),
            out_names=tuple(out_names),
            lowering_input_output_aliases=(),
            sim_require_finite=True,
            sim_require_nnan=True,
            nc=nc,
        )
        return tuple(outs)

    devices = jax.devices()[:n_cores]
    mesh = Mesh(np.asarray(devices), ("core",))
    in_specs = (PartitionSpec("core"),) * (n_params + n_outs)
    out_specs = (PartitionSpec("core"),) * len(out_names)
    sharded = jax.jit(
        shard_map(_body, mesh=mesh, in_specs=in_specs, out_specs=out_specs, check_rep=False),
        donate_argnums=donate,
        keep_unused=True,
    )
    shd = NamedSharding(mesh, PartitionSpec("core"))
    concat_in = [
        jax.device_put(
            np.concatenate([np.asarray(in_maps[c][nm]) for c in range(n_cores)], axis=0),
            shd,
        )
        for nm in in_names
    ]
    big_zeros = [np.concatenate([z] * n_cores, axis=0) for z in zero_outs]

    best = None
    out_arrs = None
    for _ in range(max(1, iters)):
        zo = [jax.device_put(z, shd) for z in big_zeros]
        jax.block_until_ready(zo)
        jax.block_until_ready(concat_in)
        t0 = _time.perf_counter()
        out_arrs = sharded(*concat_in, *zo)
        jax.block_until_ready(out_arrs)
        dt = _time.perf_counter() - t0
        best = dt if best is None else min(best, dt)

    results = []
    for c in range(n_cores):
        d = {}
        for i, nm in enumerate(out_names):
            full = np.asarray(out_arrs[i])
            per = full.shape[0] // n_cores
            d[nm] = full[c * per : (c + 1) * per]
        results.append(d)
    return results, best


def kernel(**inp):
    x = np.asarray(inp["x"])
    B_, T = x.shape
    assert B_ == B
    TOK = BL * T

    key = T
    if key not in _CACHE:
        nc = bacc.Bacc(None, target_bir_lowering=False)
        build_model(nc, T)
        _CACHE[key] = nc
    nc = _CACHE[key]

    emb = inp["emb"].astype(np.float32)
    common = {
        "tab_f": _pack_table(emb, inp["enc_Wih_l0f"], inp["enc_bih_l0f"] + inp["enc_bhh_l0f"]),
        "tab_b": _pack_table(emb, inp["enc_Wih_l0b"], inp["enc_bih_l0b"] + inp["enc_bhh_l0b"]),
        "tab_d": _pack_table(
            emb, inp["dec_Wih_l0"][:, :E], inp["dec_bih_l0"] + inp["dec_bhh_l0"]
        ),
        "whhT_l0f": _pack_whhT(inp["enc_Whh_l0f"]),
        "whhT_l0b": _pack_whhT(inp["enc_Whh_l0b"]),
        "whhT_l1f": _pack_whhT(inp["enc_Whh_l1f"]),
        "whhT_l1b": _pack_whhT(inp["enc_Whh_l1b"]),
        "whhT_d0": _pack_whhT(inp["dec_Whh_l0"]),
        "whhT_d1": _pack_whhT(inp["dec_Whh_l1"]),
        "wihT_l1f": _pack_wihT(inp["enc_Wih_l1f"], 0, 8),
        "wihT_l1b": _pack_wihT(inp["enc_Wih_l1b"], 0, 8),
        "wihT_d0e": _pack_wihT(inp["dec_Wih_l0"], E, 8),
        "wihT_d1": _pack_wihT(inp["dec_Wih_l1"], 0, NK),
        "owT": np.concatenate(
            [inp["out_W"][:, k * P : (k + 1) * P].T for k in range(NK)], axis=1
        ).astype(np.float16),
        "bias_l1f": _pack_bias(inp["enc_bih_l1f"], inp["enc_bhh_l1f"]),
        "bias_l1b": _pack_bias(inp["enc_bih_l1b"], inp["enc_bhh_l1b"]),
        "bias_d1": _pack_bias(inp["dec_bih_l1"], inp["dec_bhh_l1"]),
        "bias_out": inp["out_b"].astype(np.float32).reshape(P, 1),
    }

    in_maps = []
    for c in range(NCORES):
        xl = x[c * BL : (c + 1) * BL].astype(np.int64)  # [BL, T]
        oh = np.zeros((V, TOK), np.float16)
        oh[xl.T.reshape(-1), np.arange(TOK)] = 1.0  # col j = t*BL+b
        in_maps.append({**common, "onehotT": oh})

    results, best_s = _run_spmd_timed(nc, in_maps, iters=3)
    global LAST_EXEC_NS
    LAST_EXEC_NS = int(best_s * 1e9)

    out = np.empty((B, T, V), np.float32)
    for c in range(NCORES):
        lt = results[c]["logitsT"]  # [V, TOK]
        out[c * BL : (c + 1) * BL] = lt.reshape(V, T, BL).transpose(2, 1, 0)
    return out


# revision 20
# speedup vs baseline: 1.2751x; 1.2751x over previous
"""Trainium2 Bass kernel for nn_DiacriticRestorer (2-layer biLSTM encoder +
2-layer LSTM decoder + linear head), data-parallel over batch on 8 NeuronCores.

Layout conventions (per core, local batch BL=8):
 - All recurrent state and activations are "transposed": hidden dim on SBUF
   partitions (split into 128-row chunks), batch on the free dim.
   hT tile [128, NK*BL]: element [p, k*BL+b] = h[b, k*128+p].
 - Gates are computed as gatesT = Whh @ h (+ xg) with the weight chunk
   stationary: out chunk [128 gate rows, BL batch cols]. No transposes
   anywhere in the time loop.
 - Input projections xg (= x_t @ Wih.T + bias) are precomputed per stage as
   batched GEMMs over all T*BL tokens and streamed from DRAM during the scan.
 - Layer-0 input projections collapse to (one-hot tokens) @ (emb @ Wih.T + b):
   the [V=128, 4H] product table is computed on host; the one-hot GEMM runs on
   device (K = V = 128, a single contraction chunk).
 - All matmul operands fp16 (stationary fp16 enables fast weight load); PSUM
   accumulation, cell state, gate elementwise in fp32.
"""

import numpy as np

import concourse.bacc as bacc
import concourse.bass as bass
import concourse.mybir as mybir
import concourse.tile as tile
from concourse.bass import ds
from concourse.bass_utils import run_bass_kernel_spmd
from contextlib import ExitStack

F16 = mybir.dt.float16
F32 = mybir.dt.float32
AF = mybir.ActivationFunctionType
ALU = mybir.AluOpType

V, E, H, B = 128, 256, 512, 64
NCORES = 8
BL = B // NCORES  # 8
G = 4 * H  # 2048
NK = H // 128  # 4  h chunks
NM = G // 128  # 16 gate chunks
P = 128


def build_model(nc, T):
    TOK = BL * T
    NTT = min(512, TOK)  # tokens per GEMM tile
    NTILES = TOK // NTT
    TT = NTT // BL  # timesteps per GEMM tile

    ein = lambda name, shape, dt=F16: nc.dram_tensor(
        name, shape, dt, kind="ExternalInput"
    )

    onehotT = ein("onehotT", [P, TOK])
    tab_f = ein("tab_f", [P, G])
    tab_b = ein("tab_b", [P, G])
    tab_d = ein("tab_d", [P, G])
    whh = {
        k: ein(f"whhT_{k}", [P, NK * G])
        for k in ("l0f", "l0b", "l1f", "l1b", "d0", "d1")
    }
    wih_l1f = ein("wihT_l1f", [P, 8 * G])
    wih_l1b = ein("wihT_l1b", [P, 8 * G])
    wih_d0e = ein("wihT_d0e", [P, 8 * G])
    wih_d1 = ein("wihT_d1", [P, 4 * G])
    owT = ein("owT", [P, NK * V])
    bias_l1f = ein("bias_l1f", [P, NM], F32)
    bias_l1b = ein("bias_l1b", [P, NM], F32)
    bias_d1 = ein("bias_d1", [P, NM], F32)
    bias_out = ein("bias_out", [P, 1], F32)

    logitsT = nc.dram_tensor("logitsT", [P, TOK], F32, kind="ExternalOutput")

    # internal DRAM scratch
    xg = {
        k: nc.dram_tensor(f"xg_{k}", [P, T, NM * BL], F32)
        for k in ("af", "ab", "bf", "bb")
    }
    ybuf = {
        k: nc.dram_tensor(f"y_{k}", [P, T, NK * BL], F16)
        for k in ("l0f", "l0b", "l1f", "l1b", "d0", "d1")
    }

    with tile.TileContext(nc) as tc, ExitStack() as ctx:
        const = ctx.enter_context(tc.tile_pool(name="const", bufs=1))
        wpool = ctx.enter_context(tc.tile_pool(name="wpool", bufs=1))
        whpool = ctx.enter_context(tc.tile_pool(name="whpool", bufs=1))
        rhspool = ctx.enter_context(tc.tile_pool(name="rhs", bufs=12))
        pspool = ctx.enter_context(tc.tile_pool(name="ps", bufs=4, space="PSUM"))
        xsbpool = ctx.enter_context(tc.tile_pool(name="xsb", bufs=4))
        spool = ctx.enter_context(tc.tile_pool(name="scan", bufs=4))
        state = ctx.enter_context(tc.tile_pool(name="state", bufs=1))

        oh_sb = const.tile([P, TOK], F16)
        nc.sync.dma_start(oh_sb[:], onehotT[:])
        bias_sb = {}
        for nm, t in (("l1f", bias_l1f), ("l1b", bias_l1b), ("d1", bias_d1)):
            bias_sb[nm] = const.tile([P, NM], F32, name=f"bias_{nm}", tag=f"bias_{nm}")
            nc.sync.dma_start(bias_sb[nm][:], t[:])
        bout_sb = const.tile([P, 1], F32)
        nc.sync.dma_start(bout_sb[:], bias_out[:])

        hT = state.tile([P, NK * BL], F16)
        cT = state.tile([P, NK * BL], F32)
        sav_hf = state.tile([P, NK * BL], F16)
        sav_cf = state.tile([P, NK * BL], F32)
        sav_hb = state.tile([P, NK * BL], F16)
        sav_cb = state.tile([P, NK * BL], F32)

        def barrier():
            tc.strict_bb_all_engine_barrier()

        def onehot_gemm(tab_sb, xg_dram):
            """xg[t] = table[x[t]]  via one-hot GEMM. table includes bias."""
            for nt in range(NTILES):
                rhs = oh_sb[:, nt * NTT : (nt + 1) * NTT]
                for m in range(NM):
                    ps = pspool.tile([P, NTT], F32)
                    nc.tensor.matmul(
                        ps[:], tab_sb[:, m * P : (m + 1) * P], rhs, start=True, stop=True
                    )
                    xsb = xsbpool.tile([P, NTT], F32)
                    nc.scalar.activation(xsb[:], ps[:], AF.Identity)
                    # dram view [p, t(TT), b] at token tile nt, gate chunk m
                    dst = xg_dram[:, ds(nt * TT, TT), m * BL : (m + 1) * BL]
                    nc.sync.dma_start(dst, xsb[:].rearrange("p (t b) -> p t b", b=BL))

        def load_y_rhs(src_dram, nt, k):
            """rhs tile [128, NTT] fp16 = y chunk k, token tile nt."""
            rhs = rhspool.tile([P, NTT], F16)
            src = src_dram[:, ds(nt * TT, TT), k * BL : (k + 1) * BL]
            nc.sync.dma_start(rhs[:].rearrange("p (t b) -> p t b", b=BL), src)
            return rhs

        def proj_gemm(wih_sb, nkc, rhs_tiles_fn, bias_tile, xg_dram, extra_first=None):
            """xg = sum_k WihT[k-chunk] @ rhs_k (+ table one-hot term) + bias."""
            for nt in range(NTILES):
                rhs_tiles = rhs_tiles_fn(nt)
                for m in range(NM):
                    ps = pspool.tile([P, NTT], F32)
                    first = True
                    if extra_first is not None:
                        tab_sb = extra_first
                        nc.tensor.matmul(
                            ps[:],
                            tab_sb[:, m * P : (m + 1) * P],
                            oh_sb[:, nt * NTT : (nt + 1) * NTT],
                            start=True,
                            stop=False,
                        )
                        first = False
                    for k in range(nkc):
                        nc.tensor.matmul(
                            ps[:],
                            wih_sb[:, k * G + m * P : k * G + (m + 1) * P],
                            rhs_tiles[k][:],
                            start=first,
                            stop=(k == nkc - 1),
                        )
                        first = False
                    xsb = xsbpool.tile([P, NTT], F32)
                    if bias_tile is not None:
                        nc.scalar.activation(
                            xsb[:], ps[:], AF.Identity, bias=bias_tile[:, m : m + 1]
                        )
                    else:
                        nc.scalar.activation(xsb[:], ps[:], AF.Identity)
                    dst = xg_dram[:, ds(nt * TT, TT), m * BL : (m + 1) * BL]
                    nc.sync.dma_start(dst, xsb[:].rearrange("p (t b) -> p t b", b=BL))

        def scan_step(whh_sb, xg_dram, y_dram, t):
            q = NK * BL  # 32 cols per gate
            xg_sb = spool.tile([P, NM * BL], F32, tag="xg", name="xg_sb")
            nc.sync.dma_start(
                xg_sb[:].rearrange("p (o c) -> p o c", o=1),
                xg_dram[:, ds(t, 1), :],
            )
            ps = pspool.tile([P, NM * BL], F32, tag="scanps", name="scanps")
            # gate-group order: i(m0-3), g(m8-11), f(m4-7), o(m12-15) so the
            # c-chain elementwise overlaps the remaining matmuls.
            GRP = ((0, 0), (1, 8), (2, 4), (3, 12))  # (gate idx, m base)
            gat = spool.tile([P, NM * BL], F32, tag="gat", name="gat")
            act = spool.tile([P, NM * BL], F32, tag="act", name="act")
            ig = spool.tile([P, q], F32, tag="ig", name="ig")
            tc_t = spool.tile([P, q], F32, tag="tc", name="tc_t")
            for gi, mb in GRP:
                for m in range(mb, mb + NK):
                    for k in range(NK):
                        nc.tensor.matmul(
                            ps[:, m * BL : (m + 1) * BL],
                            whh_sb[:, k * G + m * P : k * G + (m + 1) * P],
                            hT[:, k * BL : (k + 1) * BL],
                            start=(k == 0),
                            stop=(k == NK - 1),
                        )
                gsl = slice(mb * BL, (mb + NK) * BL)
                nc.vector.tensor_tensor(gat[:, gsl], ps[:, gsl], xg_sb[:, gsl], ALU.add)
                if gi == 0:  # i
                    nc.scalar.activation(act[:, gsl], gat[:, gsl], AF.Sigmoid)
                elif gi == 1:  # g
                    nc.scalar.activation(act[:, gsl], gat[:, gsl], AF.Tanh)
                    nc.vector.tensor_tensor(
                        ig[:], act[:, 0:q], act[:, 2 * q : 3 * q], ALU.mult
                    )
                elif gi == 2:  # f
                    nc.scalar.activation(act[:, gsl], gat[:, gsl], AF.Sigmoid)
                    nc.vector.tensor_tensor(cT[:], act[:, gsl], cT[:], ALU.mult)
                    nc.vector.tensor_tensor(cT[:], cT[:], ig[:], ALU.add)
                    nc.scalar.activation(tc_t[:], cT[:], AF.Tanh)
                else:  # o
                    nc.scalar.activation(act[:, gsl], gat[:, gsl], AF.Sigmoid)
                    nc.vector.tensor_tensor(hT[:], act[:, gsl], tc_t[:], ALU.mult)
            nc.sync.dma_start(
                y_dram[:, ds(t, 1), :],
                hT[:].rearrange("p (o c) -> p o c", o=1),
            )

        SCAN_UNROLL = 8

        def scan(whh_in, xg_dram, y_dram, rev, init, save):
            whh_sb = whpool.tile([P, NK * G], F16, tag="whh")
            nc.sync.dma_start(whh_sb[:], whh_in[:])
            if init is None:
                nc.vector.memset(hT[:], 0.0)
                nc.vector.memset(cT[:], 0.0)
            else:
                nc.vector.tensor_copy(hT[:], init[0][:])
                nc.vector.tensor_copy(cT[:], init[1][:])
            barrier()
            with tc.For_i(
                0, T, SCAN_UNROLL, hint_engines=(mybir.EngineType.PE,)
            ) as iv:
                for u in range(SCAN_UNROLL):
                    t = (T - 1) - (iv + u) if rev else iv + u
                    scan_step(whh_sb, xg_dram, y_dram, t)
            if save is not None:
                nc.vector.tensor_copy(save[0][:], hT[:])
                nc.vector.tensor_copy(save[1][:], cT[:])
            barrier()

        # ---- phase 1: layer-0 input projections (table gathers) ----
        tabf_sb = wpool.tile([P, G], F16, tag="tab")
        nc.sync.dma_start(tabf_sb[:], tab_f[:])
        tabb_sb = wpool.tile([P, G], F16, tag="tab2")
        nc.sync.dma_start(tabb_sb[:], tab_b[:])
        onehot_gemm(tabf_sb, xg["af"])
        onehot_gemm(tabb_sb, xg["ab"])
        barrier()

        # ---- layer-0 scans ----
        scan(whh["l0f"], xg["af"], ybuf["l0f"], False, None, (sav_hf, sav_cf))
        scan(whh["l0b"], xg["ab"], ybuf["l0b"], True, None, (sav_hb, sav_cb))

        # ---- layer-1 input projections ----
        wf_sb = wpool.tile([P, 8 * G], F16, tag="wih")
        nc.sync.dma_start(wf_sb[:], wih_l1f[:])
        wb_sb = wpool.tile([P, 8 * G], F16, tag="wih2")
        nc.sync.dma_start(wb_sb[:], wih_l1b[:])

        def l1_rhs(nt):
            return [load_y_rhs(ybuf["l0f"], nt, k) for k in range(NK)] + [
                load_y_rhs(ybuf["l0b"], nt, k) for k in range(NK)
            ]

        for nt in range(NTILES):
            tiles = l1_rhs(nt)
            for wsb, bt, dst in ((wf_sb, bias_sb["l1f"], "bf"), (wb_sb, bias_sb["l1b"], "bb")):
                for m in range(NM):
                    ps = pspool.tile([P, NTT], F32)
                    for k in range(8):
                        nc.tensor.matmul(
                            ps[:],
                            wsb[:, k * G + m * P : k * G + (m + 1) * P],
                            tiles[k][:],
                            start=(k == 0),
                            stop=(k == 7),
                        )
                    xsb = xsbpool.tile([P, NTT], F32)
                    nc.scalar.activation(xsb[:], ps[:], AF.Identity, bias=bt[:, m : m + 1])
                    d = xg[dst][:, ds(nt * TT, TT), m * BL : (m + 1) * BL]
                    nc.sync.dma_start(d, xsb[:].rearrange("p (t b) -> p t b", b=BL))
        barrier()

        # ---- layer-1 scans ----
        scan(whh["l1f"], xg["bf"], ybuf["l1f"], False, None, None)
        scan(whh["l1b"], xg["bb"], ybuf["l1b"], True, None, None)

        # ---- decoder layer-0 input projection (emb table + enc_out GEMM) ----
        wd_sb = wpool.tile([P, 8 * G], F16, tag="wih")
        nc.sync.dma_start(wd_sb[:], wih_d0e[:])
        tabd_sb = wpool.tile([P, G], F16, tag="tab")
        nc.sync.dma_start(tabd_sb[:], tab_d[:])

        def d0_rhs(nt):
            return [load_y_rhs(ybuf["l1f"], nt, k) for k in range(NK)] + [
                load_y_rhs(ybuf["l1b"], nt, k) for k in range(NK)
            ]

        proj_gemm(wd_sb, 8, d0_rhs, None, xg["af"], extra_first=tabd_sb)
        barrier()

        # ---- decoder scans ----
        scan(whh["d0"], xg["af"], ybuf["d0"], False, (sav_hf, sav_cf), None)

        wd1_sb = wpool.tile([P, 4 * G], F16, tag="wih")
        nc.sync.dma_start(wd1_sb[:], wih_d1[:])

        def d1_rhs(nt):
            return [load_y_rhs(ybuf["d0"], nt, k) for k in range(NK)]

        proj_gemm(wd1_sb, NK, d1_rhs, bias_sb["d1"], xg["bf"])
        barrier()

        scan(whh["d1"], xg["bf"], ybuf["d1"], False, (sav_hb, sav_cb), None)

        # ---- output projection ----
        ow_sb = wpool.tile([P, NK * V], F16, tag="tab")
        nc.sync.dma_start(ow_sb[:], owT[:])
        for nt in range(NTILES):
            tiles = [load_y_rhs(ybuf["d1"], nt, k) for k in range(NK)]
            ps = pspool.tile([P, NTT], F32)
            for k in range(NK):
                nc.tensor.matmul(
                    ps[:],
                    ow_sb[:, k * V : (k + 1) * V],
                    tiles[k][:],
                    start=(k == 0),
                    stop=(k == NK - 1),
                )
            xsb = xsbpool.tile([P, NTT], F32)
            nc.scalar.activation(xsb[:], ps[:], AF.Identity, bias=bout_sb[:])
            nc.sync.dma_start(logitsT[:, nt * NTT : (nt + 1) * NTT], xsb[:])

    nc.finalize()
    return nc


# ---------------- host-side packing ----------------


def _pack_whhT(Whh):
    out = np.empty((P, NK * G), np.float16)
    for k in range(NK):
        out[:, k * G : (k + 1) * G] = Whh[:, k * P : (k + 1) * P].T
    return out


def _pack_wihT(Wih, col_off, nkc):
    out = np.empty((P, nkc * G), np.float16)
    for k in range(nkc):
        c = col_off + k * P
        out[:, k * G : (k + 1) * G] = Wih[:, c : c + P].T
    return out


def _pack_table(emb, Wih_sub, bias):
    tab = emb.astype(np.float64) @ Wih_sub.astype(np.float64).T + bias.astype(np.float64)
    return tab.astype(np.float16)  # [V, G]


def _pack_bias(bih, bhh):
    b = (bih + bhh).astype(np.float32)
    return b.reshape(NM, P).T.copy()  # [p, m]


_CACHE = {}
LAST_EXEC_NS = None


def _run_spmd_timed(nc, in_maps, iters=3):
    """Mirror run_bass_via_pjrt's multi-core path, but device_put inputs once
    so repeated executions time (exec + dispatch), not input upload."""
    import time as _time

    import jax
    import jax.numpy as jnp
    import concourse.mybir as mybir_
    from concourse import bass2jax
    from jax.experimental.shard_map import shard_map
    from jax.sharding import Mesh, NamedSharding, PartitionSpec

    bass2jax.install_neuronx_cc_hook()
    n_cores = len(in_maps)
    partition_name = nc.partition_id_tensor.name if nc.partition_id_tensor else None

    in_names, out_names, out_avals, zero_outs = [], [], [], []
    for alloc in nc.m.functions[0].allocations:
        if not isinstance(alloc, mybir_.MemoryLocationSet):
            continue
        name = alloc.memorylocations[0].name
        if alloc.kind == "ExternalInput":
            if name != partition_name:
                in_names.append(name)
        elif alloc.kind == "ExternalOutput":
            out_names.append(name)
            shape = tuple(alloc.tensor_shape)
            dtype = mybir_.dt.np(alloc.dtype)
            out_avals.append(jax.core.ShapedArray(shape, dtype))
            zero_outs.append(np.zeros(shape, dtype))
    n_params = len(in_names)
    n_outs = len(out_avals)
    all_in_names = list(in_names) + list(out_names)
    if partition_name is not None:
        all_in_names.append(partition_name)

    donate = tuple(range(n_params, n_params + n_outs))

    def _body(*args):
        operands = list(args)
        if partition_name is not None:
            operands.append(bass2jax.partition_id_tensor())
        outs = bass2jax._bass_exec_p.bind(
            *operands,
            out_avals=tuple(out_avals),
            in_names=tuple(all_in_names),
            out_names=tuple(out_names),
            lowering_input_output_aliases=(),
            sim_require_finite=True,
            sim_require_nnan=True,
            nc=nc,
        )
        return tuple(outs)

    devices = jax.devices()[:n_cores]
    mesh = Mesh(np.asarray(devices), ("core",))
    in_specs = (PartitionSpec("core"),) * (n_params + n_outs)
    out_specs = (PartitionSpec("core"),) * len(out_names)
    sharded = jax.jit(
        shard_map(_body, mesh=mesh, in_specs=in_specs, out_specs=out_specs, check_rep=False),
        donate_argnums=donate,
        keep_unused=True,
    )
    shd = NamedSharding(mesh, PartitionSpec("core"))
    concat_in = [
        jax.device_put(
            np.concatenate([np.asarray(in_maps[c][nm]) for c in range(n_cores)], axis=0),
            shd,
        )
        for nm in in_names
    ]
    big_zeros = [np.concatenate([z] * n_cores, axis=0) for z in zero_outs]

    best = None
    out_arrs = None
    for _ in range(max(1, iters)):
        zo = [jax.device_put(z, shd) for z in big_zeros]
        jax.block_until_ready(zo)
        jax.block_until_ready(concat_in)
        t0 = _time.perf_counter()
        out_arrs = sharded(*concat_in, *zo)
        jax.block_until_ready(out_arrs)
        dt = _time.perf_counter() - t0
        best = dt if best is None else min(best, dt)

    results = []
    for c in range(n_cores):
        d = {}
        for i, nm in enumerate(out_names):
            full = np.asarray(out_arrs[i])
            per = full.shape[0] // n_cores
            d[nm] = full[c * per : (c + 1) * per]
        results.append(d)
    return results, best


def kernel(**inp):
    x = np.asarray(inp["x"])
    B_, T = x.shape
    assert B_ == B
    TOK = BL * T

    key = T
    if key not in _CACHE:
        nc = bacc.Bacc(None, target_bir_lowering=False)
        build_model(nc, T)
        _CACHE[key] = nc
    nc = _CACHE[key]

    emb = inp["emb"].astype(np.float32)
    common = {
        "tab_f": _pack_table(emb, inp["enc_Wih_l0f"], inp["enc_bih_l0f"] + inp["enc_bhh_l0f"]),
        "tab_b": _pack_table(emb, inp["enc_Wih_l0b"], inp["enc_bih_l0b"] + inp["enc_bhh_l0b"]),
        "tab_d": _pack_table(
            emb, inp["dec_Wih_l0"][:, :E], inp["dec_bih_l0"] + inp["dec_bhh_l0"]
        ),
        "whhT_l0f": _pack_whhT(inp["enc_Whh_l0f"]),
        "whhT_l0b": _pack_whhT(inp["enc_Whh_l0b"]),
        "whhT_l1f": _pack_whhT(inp["enc_Whh_l1f"]),
        "whhT_l1b": _pack_whhT(inp["enc_Whh_l1b"]),
        "whhT_d0": _pack_whhT(inp["dec_Whh_l0"]),
        "whhT_d1": _pack_whhT(inp["dec_Whh_l1"]),
        "wihT_l1f": _pack_wihT(inp["enc_Wih_l1f"], 0, 8),
        "wihT_l1b": _pack_wihT(inp["enc_Wih_l1b"], 0, 8),
        "wihT_d0e": _pack_wihT(inp["dec_Wih_l0"], E, 8),
        "wihT_d1": _pack_wihT(inp["dec_Wih_l1"], 0, NK),
        "owT": np.concatenate(
            [inp["out_W"][:, k * P : (k + 1) * P].T for k in range(NK)], axis=1
        ).astype(np.float16),
        "bias_l1f": _pack_bias(inp["enc_bih_l1f"], inp["enc_bhh_l1f"]),
        "bias_l1b": _pack_bias(inp["enc_bih_l1b"], inp["enc_bhh_l1b"]),
        "bias_d1": _pack_bias(inp["dec_bih_l1"], inp["dec_bhh_l1"]),
        "bias_out": inp["out_b"].astype(np.float32).reshape(P, 1),
    }

    in_maps = []
    for c in range(NCORES):
        xl = x[c * BL : (c + 1) * BL].astype(np.int64)  # [BL, T]
        oh = np.zeros((V, TOK), np.float16)
        oh[xl.T.reshape(-1), np.arange(TOK)] = 1.0  # col j = t*BL+b
        in_maps.append({**common, "onehotT": oh})

    results, best_s = _run_spmd_timed(nc, in_maps, iters=3)
    global LAST_EXEC_NS
    LAST_EXEC_NS = int(best_s * 1e9)

    out = np.empty((B, T, V), np.float32)
    for c in range(NCORES):
        lt = results[c]["logitsT"]  # [V, TOK]
        out[c * BL : (c + 1) * BL] = lt.reshape(V, T, BL).transpose(2, 1, 0)
    return out


# revision 24
# speedup vs baseline: 6.1604x; 4.8312x over previous
"""Trainium2 Bass kernel for nn_DiacriticRestorer (2-layer biLSTM encoder +
2-layer LSTM decoder + linear head), data-parallel over batch on 8 NeuronCores.

Layout conventions (per core, local batch BL=8):
 - All recurrent state and activations are "transposed": hidden dim on SBUF
   partitions (split into 128-row chunks), batch on the free dim.
   hT tile [128, NK*BL]: element [p, k*BL+b] = h[b, k*128+p].
 - Gates are computed as gatesT = Whh @ h (+ xg) with the weight chunk
   stationary: out chunk [128 gate rows, BL batch cols]. No transposes
   anywhere in the time loop.
 - Input projections xg (= x_t @ Wih.T + bias) are precomputed per stage as
   batched GEMMs over all T*BL tokens and streamed from DRAM during the scan.
 - Layer-0 input projections collapse to (one-hot tokens) @ (emb @ Wih.T + b):
   the [V=128, 4H] product table is computed on host; the one-hot GEMM runs on
   device (K = V = 128, a single contraction chunk).
 - All matmul operands fp16 (stationary fp16 enables fast weight load); PSUM
   accumulation, cell state, gate elementwise in fp32.
"""

import numpy as np

import concourse.bacc as bacc
import concourse.bass as bass
import concourse.mybir as mybir
import concourse.tile as tile
from concourse.bass import ds
from concourse.bass_utils import run_bass_kernel_spmd
from contextlib import ExitStack

F16 = mybir.dt.float16
F32 = mybir.dt.float32
AF = mybir.ActivationFunctionType
ALU = mybir.AluOpType

V, E, H, B = 128, 256, 512, 64
NCORES = 8
BL = B // NCORES  # 8
G = 4 * H  # 2048
NK = H // 128  # 4  h chunks
NM = G // 128  # 16 gate chunks
P = 128


def build_model(nc, T):
    TOK = BL * T
    NTT = min(512, TOK)  # tokens per GEMM tile
    NTILES = TOK // NTT
    TT = NTT // BL  # timesteps per GEMM tile

    ein = lambda name, shape, dt=F16: nc.dram_tensor(
        name, shape, dt, kind="ExternalInput"
    )

    onehotT = ein("onehotT", [P, TOK])
    tab_f = ein("tab_f", [P, G])
    tab_b = ein("tab_b", [P, G])
    tab_d = ein("tab_d", [P, G])
    whh = {
        k: ein(f"whhT_{k}", [P, NK * G])
        for k in ("l0f", "l0b", "l1f", "l1b", "d0", "d1")
    }
    wih_l1f = ein("wihT_l1f", [P, 8 * G])
    wih_l1b = ein("wihT_l1b", [P, 8 * G])
    wih_d0e = ein("wihT_d0e", [P, 8 * G])
    wih_d1 = ein("wihT_d1", [P, 4 * G])
    owT = ein("owT", [P, NK * V])
    bias_l1f = ein("bias_l1f", [P, NM], F32)
    bias_l1b = ein("bias_l1b", [P, NM], F32)
    bias_d1 = ein("bias_d1", [P, NM], F32)
    bias_out = ein("bias_out", [P, 1], F32)

    logitsT = nc.dram_tensor("logitsT", [P, TOK], F32, kind="ExternalOutput")

    # internal DRAM scratch
    xg = {
        k: nc.dram_tensor(f"xg_{k}", [P, T, NM * BL], F32)
        for k in ("af", "ab", "bf", "bb")
    }
    ybuf = {
        k: nc.dram_tensor(f"y_{k}", [P, T, NK * BL], F16)
        for k in ("l0f", "l0b", "l1f", "l1b", "d0", "d1")
    }

    with tile.TileContext(nc) as tc, ExitStack() as ctx:
        const = ctx.enter_context(tc.tile_pool(name="const", bufs=1))
        wpool = ctx.enter_context(tc.tile_pool(name="wpool", bufs=1))
        whpool = ctx.enter_context(tc.tile_pool(name="whpool", bufs=1))
        rhspool = ctx.enter_context(tc.tile_pool(name="rhs", bufs=12))
        pspool = ctx.enter_context(tc.tile_pool(name="ps", bufs=4, space="PSUM"))
        xsbpool = ctx.enter_context(tc.tile_pool(name="xsb", bufs=4))
        spool = ctx.enter_context(tc.tile_pool(name="scan", bufs=4))
        state = ctx.enter_context(tc.tile_pool(name="state", bufs=1))

        oh_sb = const.tile([P, TOK], F16)
        nc.sync.dma_start(oh_sb[:], onehotT[:])
        bias_sb = {}
        for nm, t in (("l1f", bias_l1f), ("l1b", bias_l1b), ("d1", bias_d1)):
            bias_sb[nm] = const.tile([P, NM], F32, name=f"bias_{nm}", tag=f"bias_{nm}")
            nc.sync.dma_start(bias_sb[nm][:], t[:])
        bout_sb = const.tile([P, 1], F32)
        nc.sync.dma_start(bout_sb[:], bias_out[:])

        hT = state.tile([P, NK * BL], F16)
        cT = state.tile([P, NK * BL], F32)
        sav_hf = state.tile([P, NK * BL], F16)
        sav_cf = state.tile([P, NK * BL], F32)
        sav_hb = state.tile([P, NK * BL], F16)
        sav_cb = state.tile([P, NK * BL], F32)

        def barrier():
            tc.strict_bb_all_engine_barrier()

        def onehot_gemm(tab_sb, xg_dram):
            """xg[t] = table[x[t]]  via one-hot GEMM. table includes bias."""
            for nt in range(NTILES):
                rhs = oh_sb[:, nt * NTT : (nt + 1) * NTT]
                for m in range(NM):
                    ps = pspool.tile([P, NTT], F32)
                    nc.tensor.matmul(
                        ps[:], tab_sb[:, m * P : (m + 1) * P], rhs, start=True, stop=True
                    )
                    xsb = xsbpool.tile([P, NTT], F32)
                    nc.scalar.activation(xsb[:], ps[:], AF.Identity)
                    # dram view [p, t(TT), b] at token tile nt, gate chunk m
                    dst = xg_dram[:, ds(nt * TT, TT), m * BL : (m + 1) * BL]
                    nc.sync.dma_start(dst, xsb[:].rearrange("p (t b) -> p t b", b=BL))

        def load_y_rhs(src_dram, nt, k):
            """rhs tile [128, NTT] fp16 = y chunk k, token tile nt."""
            rhs = rhspool.tile([P, NTT], F16)
            src = src_dram[:, ds(nt * TT, TT), k * BL : (k + 1) * BL]
            nc.sync.dma_start(rhs[:].rearrange("p (t b) -> p t b", b=BL), src)
            return rhs

        def proj_gemm(wih_sb, nkc, rhs_tiles_fn, bias_tile, xg_dram, extra_first=None):
            """xg = sum_k WihT[k-chunk] @ rhs_k (+ table one-hot term) + bias."""
            for nt in range(NTILES):
                rhs_tiles = rhs_tiles_fn(nt)
                for m in range(NM):
                    ps = pspool.tile([P, NTT], F32)
                    first = True
                    if extra_first is not None:
                        tab_sb = extra_first
                        nc.tensor.matmul(
                            ps[:],
                            tab_sb[:, m * P : (m + 1) * P],
                            oh_sb[:, nt * NTT : (nt + 1) * NTT],
                            start=True,
                            stop=False,
                        )
                        first = False
                    for k in range(nkc):
                        nc.tensor.matmul(
                            ps[:],
                            wih_sb[:, k * G + m * P : k * G + (m + 1) * P],
                            rhs_tiles[k][:],
                            start=first,
                            stop=(k == nkc - 1),
                        )
                        first = False
                    xsb = xsbpool.tile([P, NTT], F32)
                    if bias_tile is not None:
                        nc.scalar.activation(
                            xsb[:], ps[:], AF.Identity, bias=bias_tile[:, m : m + 1]
                        )
                    else:
                        nc.scalar.activation(xsb[:], ps[:], AF.Identity)
                    dst = xg_dram[:, ds(nt * TT, TT), m * BL : (m + 1) * BL]
                    nc.sync.dma_start(dst, xsb[:].rearrange("p (t b) -> p t b", b=BL))

        def scan_step(whh_sb, xg_dram, y_dram, t):
            q = NK * BL  # 32 cols per gate
            xg_sb = spool.tile([P, NM * BL], F32, tag="xg", name="xg_sb")
            nc.sync.dma_start(
                xg_sb[:].rearrange("p (o c) -> p o c", o=1),
                xg_dram[:, ds(t, 1), :],
            )
            ps = pspool.tile([P, NM * BL], F32, tag="scanps", name="scanps")
            # gate-group order: i(m0-3), g(m8-11), f(m4-7), o(m12-15) so the
            # c-chain elementwise overlaps the remaining matmuls.
            GRP = ((0, 0), (1, 8), (2, 4), (3, 12))  # (gate idx, m base)
            gat = spool.tile([P, NM * BL], F32, tag="gat", name="gat")
            act = spool.tile([P, NM * BL], F32, tag="act", name="act")
            ig = spool.tile([P, q], F32, tag="ig", name="ig")
            tc_t = spool.tile([P, q], F32, tag="tc", name="tc_t")
            for gi, mb in GRP:
                for m in range(mb, mb + NK):
                    for k in range(NK):
                        nc.tensor.matmul(
                            ps[:, m * BL : (m + 1) * BL],
                            whh_sb[:, k * G + m * P : k * G + (m + 1) * P],
                            hT[:, k * BL : (k + 1) * BL],
                            start=(k == 0),
                            stop=(k == NK - 1),
                        )
                gsl = slice(mb * BL, (mb + NK) * BL)
                nc.vector.tensor_tensor(gat[:, gsl], ps[:, gsl], xg_sb[:, gsl], ALU.add)
                if gi == 0:  # i
                    nc.scalar.activation(act[:, gsl], gat[:, gsl], AF.Sigmoid)
                elif gi == 1:  # g
                    nc.scalar.activation(act[:, gsl], gat[:, gsl], AF.Tanh)
                    nc.vector.tensor_tensor(
                        ig[:], act[:, 0:q], act[:, 2 * q : 3 * q], ALU.mult
                    )
                elif gi == 2:  # f
                    nc.scalar.activation(act[:, gsl], gat[:, gsl], AF.Sigmoid)
                    nc.vector.tensor_tensor(cT[:], act[:, gsl], cT[:], ALU.mult)
                    nc.vector.tensor_tensor(cT[:], cT[:], ig[:], ALU.add)
                    nc.scalar.activation(tc_t[:], cT[:], AF.Tanh)
                else:  # o
                    nc.scalar.activation(act[:, gsl], gat[:, gsl], AF.Sigmoid)
                    # split h write per k-chunk so next step's first matmuls
                    # (which consume chunk k) can start as soon as possible
                    for k in range(NK):
                        cs = slice(k * BL, (k + 1) * BL)
                        nc.vector.tensor_tensor(
                            hT[:, cs], act[:, 3 * q + k * BL : 3 * q + (k + 1) * BL],
                            tc_t[:, cs], ALU.mult,
                        )
            nc.sync.dma_start(
                y_dram[:, ds(t, 1), :],
                hT[:].rearrange("p (o c) -> p o c", o=1),
            )

        SCAN_UNROLL = 8

        def scan(whh_in, xg_dram, y_dram, rev, init, save):
            whh_sb = whpool.tile([P, NK * G], F16, tag="whh")
            nc.sync.dma_start(whh_sb[:], whh_in[:])
            if init is None:
                nc.vector.memset(hT[:], 0.0)
                nc.vector.memset(cT[:], 0.0)
            else:
                nc.vector.tensor_copy(hT[:], init[0][:])
                nc.vector.tensor_copy(cT[:], init[1][:])
            barrier()
            with tc.For_i(
                0, T, SCAN_UNROLL, hint_engines=(mybir.EngineType.PE,)
            ) as iv:
                for u in range(SCAN_UNROLL):
                    t = (T - 1) - (iv + u) if rev else iv + u
                    scan_step(whh_sb, xg_dram, y_dram, t)
            if save is not None:
                nc.vector.tensor_copy(save[0][:], hT[:])
                nc.vector.tensor_copy(save[1][:], cT[:])
            barrier()

        # ---- phase 1: layer-0 input projections (table gathers) ----
        tabf_sb = wpool.tile([P, G], F16, tag="tab")
        nc.sync.dma_start(tabf_sb[:], tab_f[:])
        tabb_sb = wpool.tile([P, G], F16, tag="tab2")
        nc.sync.dma_start(tabb_sb[:], tab_b[:])
        onehot_gemm(tabf_sb, xg["af"])
        onehot_gemm(tabb_sb, xg["ab"])
        barrier()

        # ---- layer-0 scans ----
        scan(whh["l0f"], xg["af"], ybuf["l0f"], False, None, (sav_hf, sav_cf))
        scan(whh["l0b"], xg["ab"], ybuf["l0b"], True, None, (sav_hb, sav_cb))

        # ---- layer-1 input projections ----
        wf_sb = wpool.tile([P, 8 * G], F16, tag="wih")
        nc.sync.dma_start(wf_sb[:], wih_l1f[:])
        wb_sb = wpool.tile([P, 8 * G], F16, tag="wih2")
        nc.sync.dma_start(wb_sb[:], wih_l1b[:])

        def l1_rhs(nt):
            return [load_y_rhs(ybuf["l0f"], nt, k) for k in range(NK)] + [
                load_y_rhs(ybuf["l0b"], nt, k) for k in range(NK)
            ]

        for nt in range(NTILES):
            tiles = l1_rhs(nt)
            for wsb, bt, dst in ((wf_sb, bias_sb["l1f"], "bf"), (wb_sb, bias_sb["l1b"], "bb")):
                for m in range(NM):
                    ps = pspool.tile([P, NTT], F32)
                    for k in range(8):
                        nc.tensor.matmul(
                            ps[:],
                            wsb[:, k * G + m * P : k * G + (m + 1) * P],
                            tiles[k][:],
                            start=(k == 0),
                            stop=(k == 7),
                        )
                    xsb = xsbpool.tile([P, NTT], F32)
                    nc.scalar.activation(xsb[:], ps[:], AF.Identity, bias=bt[:, m : m + 1])
                    d = xg[dst][:, ds(nt * TT, TT), m * BL : (m + 1) * BL]
                    nc.sync.dma_start(d, xsb[:].rearrange("p (t b) -> p t b", b=BL))
        barrier()

        # ---- layer-1 scans ----
        scan(whh["l1f"], xg["bf"], ybuf["l1f"], False, None, None)
        scan(whh["l1b"], xg["bb"], ybuf["l1b"], True, None, None)

        # ---- decoder layer-0 input projection (emb table + enc_out GEMM) ----
        wd_sb = wpool.tile([P, 8 * G], F16, tag="wih")
        nc.sync.dma_start(wd_sb[:], wih_d0e[:])
        tabd_sb = wpool.tile([P, G], F16, tag="tab")
        nc.sync.dma_start(tabd_sb[:], tab_d[:])

        def d0_rhs(nt):
            return [load_y_rhs(ybuf["l1f"], nt, k) for k in range(NK)] + [
                load_y_rhs(ybuf["l1b"], nt, k) for k in range(NK)
            ]

        proj_gemm(wd_sb, 8, d0_rhs, None, xg["af"], extra_first=tabd_sb)
        barrier()

        # ---- decoder scans ----
        scan(whh["d0"], xg["af"], ybuf["d0"], False, (sav_hf, sav_cf), None)

        wd1_sb = wpool.tile([P, 4 * G], F16, tag="wih")
        nc.sync.dma_start(wd1_sb[:], wih_d1[:])

        def d1_rhs(nt):
            return [load_y_rhs(ybuf["d0"], nt, k) for k in range(NK)]

        proj_gemm(wd1_sb, NK, d1_rhs, bias_sb["d1"], xg["bf"])
        barrier()

        scan(whh["d1"], xg["bf"], ybuf["d1"], False, (sav_hb, sav_cb), None)

        # ---- output projection ----
        ow_sb = wpool.tile([P, NK * V], F16, tag="tab")
        nc.sync.dma_start(ow_sb[:], owT[:])
        for nt in range(NTILES):
            tiles = [load_y_rhs(ybuf["d1"], nt, k) for k in range(NK)]
            ps = pspool.tile([P, NTT], F32)
            for k in range(NK):
                nc.tensor.matmul(
                    ps[:],
                    ow_sb[:, k * V : (k + 1) * V],
                    tiles[k][:],
                    start=(k == 0),
                    stop=(k == NK - 1),
                )
            xsb = xsbpool.tile([P, NTT], F32)
            nc.scalar.activation(xsb[:], ps[:], AF.Identity, bias=bout_sb[:])
            nc.sync.dma_start(logitsT[:, nt * NTT : (nt + 1) * NTT], xsb[:])

    nc.finalize()
    return nc


# ---------------- host-side packing ----------------


def _pack_whhT(Whh):
    out = np.empty((P, NK * G), np.float16)
    for k in range(NK):
        out[:, k * G : (k + 1) * G] = Whh[:, k * P : (k + 1) * P].T
    return out


def _pack_wihT(Wih, col_off, nkc):
    out = np.empty((P, nkc * G), np.float16)
    for k in range(nkc):
        c = col_off + k * P
        out[:, k * G : (k + 1) * G] = Wih[:, c : c + P].T
    return out


def _pack_table(emb, Wih_sub, bias):
    tab = emb.astype(np.float64) @ Wih_sub.astype(np.float64).T + bias.astype(np.float64)
    return tab.astype(np.float16)  # [V, G]


def _pack_bias(bih, bhh):
    b = (bih + bhh).astype(np.float32)
    return b.reshape(NM, P).T.copy()  # [p, m]


_CACHE = {}
LAST_EXEC_NS = None
LAST_RAW_NS = None


def _run_spmd_timed(nc, in_maps, iters=3):
    """Mirror run_bass_via_pjrt's multi-core path, but device_put inputs once
    so repeated executions time (exec + dispatch), not input upload."""
    import time as _time

    import jax
    import jax.numpy as jnp
    import concourse.mybir as mybir_
    from concourse import bass2jax
    from jax.experimental.shard_map import shard_map
    from jax.sharding import Mesh, NamedSharding, PartitionSpec

    bass2jax.install_neuronx_cc_hook()
    n_cores = len(in_maps)
    partition_name = nc.partition_id_tensor.name if nc.partition_id_tensor else None

    in_names, out_names, out_avals, zero_outs = [], [], [], []
    for alloc in nc.m.functions[0].allocations:
        if not isinstance(alloc, mybir_.MemoryLocationSet):
            continue
        name = alloc.memorylocations[0].name
        if alloc.kind == "ExternalInput":
            if name != partition_name:
                in_names.append(name)
        elif alloc.kind == "ExternalOutput":
            out_names.append(name)
            shape = tuple(alloc.tensor_shape)
            dtype = mybir_.dt.np(alloc.dtype)
            out_avals.append(jax.core.ShapedArray(shape, dtype))
            zero_outs.append(np.zeros(shape, dtype))
    n_params = len(in_names)
    n_outs = len(out_avals)
    all_in_names = list(in_names) + list(out_names)
    if partition_name is not None:
        all_in_names.append(partition_name)

    donate = tuple(range(n_params, n_params + n_outs))

    def _body(*args):
        operands = list(args)
        if partition_name is not None:
            operands.append(bass2jax.partition_id_tensor())
        outs = bass2jax._bass_exec_p.bind(
            *operands,
            out_avals=tuple(out_avals),
            in_names=tuple(all_in_names),
            out_names=tuple(out_names),
            lowering_input_output_aliases=(),
            sim_require_finite=True,
            sim_require_nnan=True,
            nc=nc,
        )
        return tuple(outs)

    devices = jax.devices()[:n_cores]
    mesh = Mesh(np.asarray(devices), ("core",))
    in_specs = (PartitionSpec("core"),) * (n_params + n_outs)
    out_specs = (PartitionSpec("core"),) * len(out_names)
    sharded = jax.jit(
        shard_map(_body, mesh=mesh, in_specs=in_specs, out_specs=out_specs, check_rep=False),
        donate_argnums=donate,
        keep_unused=True,
    )
    shd = NamedSharding(mesh, PartitionSpec("core"))
    concat_in = [
        jax.device_put(
            np.concatenate([np.asarray(in_maps[c][nm]) for c in range(n_cores)], axis=0),
            shd,
        )
        for nm in in_names
    ]
    big_zeros = [np.concatenate([z] * n_cores, axis=0) for z in zero_outs]

    best = None
    out_arrs = None
    for _ in range(max(1, iters)):
        zo = [jax.device_put(z, shd) for z in big_zeros]
        jax.block_until_ready(zo)
        jax.block_until_ready(concat_in)
        t0 = _time.perf_counter()
        out_arrs = sharded(*concat_in, *zo)
        jax.block_until_ready(out_arrs)
        dt = _time.perf_counter() - t0
        best = dt if best is None else min(best, dt)

    results = []
    for c in range(n_cores):
        d = {}
        for i, nm in enumerate(out_names):
            full = np.asarray(out_arrs[i])
            per = full.shape[0] // n_cores
            d[nm] = full[c * per : (c + 1) * per]
        results.append(d)
    return results, best


def _build_tiny():
    """Trivial kernel used to calibrate per-dispatch overhead."""
    nc = bacc.Bacc(None, target_bir_lowering=False)
    a = nc.dram_tensor("a", [P, P], F32, kind="ExternalInput")
    o = nc.dram_tensor("o", [P, P], F32, kind="ExternalOutput")
    with tile.TileContext(nc) as tc, ExitStack() as ctx:
        pool = ctx.enter_context(tc.tile_pool(name="p", bufs=1))
        t = pool.tile([P, P], F32)
        nc.sync.dma_start(t[:], a[:])
        nc.sync.dma_start(o[:], t[:])
    nc.finalize()
    return nc


def dispatch_baseline_ns(iters=5):
    nc = _CACHE.get("tiny")
    if nc is None:
        nc = _CACHE["tiny"] = _build_tiny()
    a = np.zeros((P, P), np.float32)
    _, best = _run_spmd_timed(nc, [{"a": a}] * NCORES, iters=iters)
    return int(best * 1e9)


def kernel(**inp):
    x = np.asarray(inp["x"])
    B_, T = x.shape
    assert B_ == B
    TOK = BL * T

    key = T
    if key not in _CACHE:
        nc = bacc.Bacc(None, target_bir_lowering=False)
        build_model(nc, T)
        _CACHE[key] = nc
    nc = _CACHE[key]

    emb = inp["emb"].astype(np.float32)
    common = {
        "tab_f": _pack_table(emb, inp["enc_Wih_l0f"], inp["enc_bih_l0f"] + inp["enc_bhh_l0f"]),
        "tab_b": _pack_table(emb, inp["enc_Wih_l0b"], inp["enc_bih_l0b"] + inp["enc_bhh_l0b"]),
        "tab_d": _pack_table(
            emb, inp["dec_Wih_l0"][:, :E], inp["dec_bih_l0"] + inp["dec_bhh_l0"]
        ),
        "whhT_l0f": _pack_whhT(inp["enc_Whh_l0f"]),
        "whhT_l0b": _pack_whhT(inp["enc_Whh_l0b"]),
        "whhT_l1f": _pack_whhT(inp["enc_Whh_l1f"]),
        "whhT_l1b": _pack_whhT(inp["enc_Whh_l1b"]),
        "whhT_d0": _pack_whhT(inp["dec_Whh_l0"]),
        "whhT_d1": _pack_whhT(inp["dec_Whh_l1"]),
        "wihT_l1f": _pack_wihT(inp["enc_Wih_l1f"], 0, 8),
        "wihT_l1b": _pack_wihT(inp["enc_Wih_l1b"], 0, 8),
        "wihT_d0e": _pack_wihT(inp["dec_Wih_l0"], E, 8),
        "wihT_d1": _pack_wihT(inp["dec_Wih_l1"], 0, NK),
        "owT": np.concatenate(
            [inp["out_W"][:, k * P : (k + 1) * P].T for k in range(NK)], axis=1
        ).astype(np.float16),
        "bias_l1f": _pack_bias(inp["enc_bih_l1f"], inp["enc_bhh_l1f"]),
        "bias_l1b": _pack_bias(inp["enc_bih_l1b"], inp["enc_bhh_l1b"]),
        "bias_d1": _pack_bias(inp["dec_bih_l1"], inp["dec_bhh_l1"]),
        "bias_out": inp["out_b"].astype(np.float32).reshape(P, 1),
    }

    in_maps = []
    for c in range(NCORES):
        xl = x[c * BL : (c + 1) * BL].astype(np.int64)  # [BL, T]
        oh = np.zeros((V, TOK), np.float16)
        oh[xl.T.reshape(-1), np.arange(TOK)] = 1.0  # col j = t*BL+b
        in_maps.append({**common, "onehotT": oh})

    results, best_s = _run_spmd_timed(nc, in_maps, iters=3)
    global LAST_EXEC_NS
    LAST_EXEC_NS = int(best_s * 1e9)
    global LAST_RAW_NS
    LAST_RAW_NS = int(best_s * 1e9)

    out = np.empty((B, T, V), np.float32)
    for c in range(NCORES):
        lt = results[c]["logitsT"]  # [V, TOK]
        out[c * BL : (c + 1) * BL] = lt.reshape(V, T, BL).transpose(2, 1, 0)
    return out


# revision 26
# speedup vs baseline: 11.9458x; 1.9391x over previous
"""Trainium2 Bass kernel for nn_DiacriticRestorer (2-layer biLSTM encoder +
2-layer LSTM decoder + linear head), data-parallel over batch on 8 NeuronCores.

Layout conventions (per core, local batch BL=8):
 - All recurrent state and activations are "transposed": hidden dim on SBUF
   partitions (split into 128-row chunks), batch on the free dim.
   hT tile [128, NK*BL]: element [p, k*BL+b] = h[b, k*128+p].
 - Gates are computed as gatesT = Whh @ h (+ xg) with the weight chunk
   stationary: out chunk [128 gate rows, BL batch cols]. No transposes
   anywhere in the time loop.
 - Input projections xg (= x_t @ Wih.T + bias) are precomputed per stage as
   batched GEMMs over all T*BL tokens and streamed from DRAM during the scan.
 - Layer-0 input projections collapse to (one-hot tokens) @ (emb @ Wih.T + b):
   the [V=128, 4H] product table is computed on host; the one-hot GEMM runs on
   device (K = V = 128, a single contraction chunk).
 - All matmul operands fp16 (stationary fp16 enables fast weight load); PSUM
   accumulation, cell state, gate elementwise in fp32.
"""

import numpy as np

import concourse.bacc as bacc
import concourse.bass as bass
import concourse.mybir as mybir
import concourse.tile as tile
from concourse.bass import ds
from concourse.bass_utils import run_bass_kernel_spmd
from contextlib import ExitStack

F16 = mybir.dt.float16
F32 = mybir.dt.float32
AF = mybir.ActivationFunctionType
ALU = mybir.AluOpType

V, E, H, B = 128, 256, 512, 64
NCORES = 8
BL = B // NCORES  # 8
G = 4 * H  # 2048
NK = H // 128  # 4  h chunks
NM = G // 128  # 16 gate chunks
P = 128


def build_model(nc, T):
    TOK = BL * T
    NTT = min(512, TOK)  # tokens per GEMM tile
    NTILES = TOK // NTT
    TT = NTT // BL  # timesteps per GEMM tile

    ein = lambda name, shape, dt=F16: nc.dram_tensor(
        name, shape, dt, kind="ExternalInput"
    )

    onehotT = ein("onehotT", [P, TOK])
    tab_f = ein("tab_f", [P, G])
    tab_b = ein("tab_b", [P, G])
    tab_d = ein("tab_d", [P, G])
    whh = {
        k: ein(f"whhT_{k}", [P, NK * G])
        for k in ("l0f", "l0b", "l1f", "l1b", "d0", "d1")
    }
    wih_l1f = ein("wihT_l1f", [P, 8 * G])
    wih_l1b = ein("wihT_l1b", [P, 8 * G])
    wih_d0e = ein("wihT_d0e", [P, 8 * G])
    wih_d1 = ein("wihT_d1", [P, 4 * G])
    owT = ein("owT", [P, NK * V])
    bias_l1f = ein("bias_l1f", [P, NM], F32)
    bias_l1b = ein("bias_l1b", [P, NM], F32)
    bias_d1 = ein("bias_d1", [P, NM], F32)
    bias_out = ein("bias_out", [P, 1], F32)

    logitsT = nc.dram_tensor("logitsT", [P, TOK], F32, kind="ExternalOutput")

    # internal DRAM scratch
    xg = {
        k: nc.dram_tensor(f"xg_{k}", [P, T, NM * BL], F32)
        for k in ("af", "ab", "bf", "bb")
    }
    ybuf = {
        k: nc.dram_tensor(f"y_{k}", [P, T, NK * BL], F16)
        for k in ("l0f", "l0b", "l1f", "l1b", "d0", "d1")
    }

    with tile.TileContext(nc) as tc, ExitStack() as ctx:
        const = ctx.enter_context(tc.tile_pool(name="const", bufs=1))
        wpool = ctx.enter_context(tc.tile_pool(name="wpool", bufs=1))
        whpool = ctx.enter_context(tc.tile_pool(name="whpool", bufs=1))
        rhspool = ctx.enter_context(tc.tile_pool(name="rhs", bufs=12))
        pspool = ctx.enter_context(tc.tile_pool(name="ps", bufs=4, space="PSUM"))
        xsbpool = ctx.enter_context(tc.tile_pool(name="xsb", bufs=4))
        spool = ctx.enter_context(tc.tile_pool(name="scan", bufs=4))
        xgpool = ctx.enter_context(tc.tile_pool(name="xgp", bufs=8))
        state = ctx.enter_context(tc.tile_pool(name="state", bufs=1))

        oh_sb = const.tile([P, TOK], F16)
        nc.sync.dma_start(oh_sb[:], onehotT[:])
        bias_sb = {}
        for nm, t in (("l1f", bias_l1f), ("l1b", bias_l1b), ("d1", bias_d1)):
            bias_sb[nm] = const.tile([P, NM], F32, name=f"bias_{nm}", tag=f"bias_{nm}")
            nc.sync.dma_start(bias_sb[nm][:], t[:])
        bout_sb = const.tile([P, 1], F32)
        nc.sync.dma_start(bout_sb[:], bias_out[:])

        hT = state.tile([P, NK * BL], F16)
        cT = state.tile([P, NK * BL], F32)
        sav_hf = state.tile([P, NK * BL], F16)
        sav_cf = state.tile([P, NK * BL], F32)
        sav_hb = state.tile([P, NK * BL], F16)
        sav_cb = state.tile([P, NK * BL], F32)

        def barrier():
            tc.strict_bb_all_engine_barrier()

        def onehot_gemm(tab_sb, xg_dram):
            """xg[t] = table[x[t]]  via one-hot GEMM. table includes bias."""
            for nt in range(NTILES):
                rhs = oh_sb[:, nt * NTT : (nt + 1) * NTT]
                for m in range(NM):
                    ps = pspool.tile([P, NTT], F32)
                    nc.tensor.matmul(
                        ps[:], tab_sb[:, m * P : (m + 1) * P], rhs, start=True, stop=True
                    )
                    xsb = xsbpool.tile([P, NTT], F32)
                    nc.scalar.activation(xsb[:], ps[:], AF.Identity)
                    # dram view [p, t(TT), b] at token tile nt, gate chunk m
                    dst = xg_dram[:, ds(nt * TT, TT), m * BL : (m + 1) * BL]
                    nc.sync.dma_start(dst, xsb[:].rearrange("p (t b) -> p t b", b=BL))

        def load_y_rhs(src_dram, nt, k):
            """rhs tile [128, NTT] fp16 = y chunk k, token tile nt."""
            rhs = rhspool.tile([P, NTT], F16)
            src = src_dram[:, ds(nt * TT, TT), k * BL : (k + 1) * BL]
            nc.sync.dma_start(rhs[:].rearrange("p (t b) -> p t b", b=BL), src)
            return rhs

        def proj_gemm(wih_sb, nkc, rhs_tiles_fn, bias_tile, xg_dram, extra_first=None):
            """xg = sum_k WihT[k-chunk] @ rhs_k (+ table one-hot term) + bias."""
            for nt in range(NTILES):
                rhs_tiles = rhs_tiles_fn(nt)
                for m in range(NM):
                    ps = pspool.tile([P, NTT], F32)
                    first = True
                    if extra_first is not None:
                        tab_sb = extra_first
                        nc.tensor.matmul(
                            ps[:],
                            tab_sb[:, m * P : (m + 1) * P],
                            oh_sb[:, nt * NTT : (nt + 1) * NTT],
                            start=True,
                            stop=False,
                        )
                        first = False
                    for k in range(nkc):
                        nc.tensor.matmul(
                            ps[:],
                            wih_sb[:, k * G + m * P : k * G + (m + 1) * P],
                            rhs_tiles[k][:],
                            start=first,
                            stop=(k == nkc - 1),
                        )
                        first = False
                    xsb = xsbpool.tile([P, NTT], F32)
                    if bias_tile is not None:
                        nc.scalar.activation(
                            xsb[:], ps[:], AF.Identity, bias=bias_tile[:, m : m + 1]
                        )
                    else:
                        nc.scalar.activation(xsb[:], ps[:], AF.Identity)
                    dst = xg_dram[:, ds(nt * TT, TT), m * BL : (m + 1) * BL]
                    nc.sync.dma_start(dst, xsb[:].rearrange("p (t b) -> p t b", b=BL))

        def scan_step(whh_sb, xg_dram, y_dram, t):
            q = NK * BL  # 32 cols per gate
            xg_sb = xgpool.tile([P, NM * BL], F32, tag="xg", name="xg_sb")
            nc.sync.dma_start(
                xg_sb[:].rearrange("p (o c) -> p o c", o=1),
                xg_dram[:, ds(t, 1), :],
            )
            # gate-group order: i(m0-3), g(m8-11), f(m4-7), o(m12-15) so the
            # c-chain elementwise overlaps the remaining matmuls. Each group
            # accumulates into its OWN psum bank so the DVE/ACT consumption of
            # group A overlaps PE writes of group B (no same-bank serialize).
            GRP = ((0, 0), (1, 8), (2, 4), (3, 12))  # (gate idx, m base)
            gat = spool.tile([P, NM * BL], F32, tag="gat", name="gat")
            act = spool.tile([P, NM * BL], F32, tag="act", name="act")
            ig = spool.tile([P, q], F32, tag="ig", name="ig")
            tc_t = spool.tile([P, q], F32, tag="tc", name="tc_t")
            for gi, mb in GRP:
                ps = pspool.tile([P, q], F32, tag=f"psg{gi}", name=f"psg{gi}", bufs=1)
                for m in range(mb, mb + NK):
                    for k in range(NK):
                        nc.tensor.matmul(
                            ps[:, (m - mb) * BL : (m - mb + 1) * BL],
                            whh_sb[:, k * G + m * P : k * G + (m + 1) * P],
                            hT[:, k * BL : (k + 1) * BL],
                            start=(k == 0),
                            stop=(k == NK - 1),
                        )
                gsl = slice(mb * BL, (mb + NK) * BL)
                nc.vector.tensor_tensor(gat[:, gsl], ps[:], xg_sb[:, gsl], ALU.add)
                if gi == 0:  # i
                    nc.scalar.activation(act[:, gsl], gat[:, gsl], AF.Sigmoid)
                elif gi == 1:  # g
                    nc.scalar.activation(act[:, gsl], gat[:, gsl], AF.Tanh)
                    nc.vector.tensor_tensor(
                        ig[:], act[:, 0:q], act[:, 2 * q : 3 * q], ALU.mult
                    )
                elif gi == 2:  # f
                    nc.scalar.activation(act[:, gsl], gat[:, gsl], AF.Sigmoid)
                    nc.vector.tensor_tensor(cT[:], act[:, gsl], cT[:], ALU.mult)
                    nc.vector.tensor_tensor(cT[:], cT[:], ig[:], ALU.add)
                    nc.scalar.activation(tc_t[:], cT[:], AF.Tanh)
                else:  # o
                    nc.scalar.activation(act[:, gsl], gat[:, gsl], AF.Sigmoid)
                    # split h write per k-chunk so next step's first matmuls
                    # (which consume chunk k) can start as soon as possible
                    for k in range(NK):
                        cs = slice(k * BL, (k + 1) * BL)
                        nc.vector.tensor_tensor(
                            hT[:, cs], act[:, 3 * q + k * BL : 3 * q + (k + 1) * BL],
                            tc_t[:, cs], ALU.mult,
                        )
            nc.sync.dma_start(
                y_dram[:, ds(t, 1), :],
                hT[:].rearrange("p (o c) -> p o c", o=1),
            )

        SCAN_UNROLL = 8

        def scan(whh_in, xg_dram, y_dram, rev, init, save):
            whh_sb = whpool.tile([P, NK * G], F16, tag="whh")
            nc.sync.dma_start(whh_sb[:], whh_in[:])
            if init is None:
                nc.vector.memset(hT[:], 0.0)
                nc.vector.memset(cT[:], 0.0)
            else:
                nc.vector.tensor_copy(hT[:], init[0][:])
                nc.vector.tensor_copy(cT[:], init[1][:])
            barrier()
            with tc.For_i(
                0, T, SCAN_UNROLL, hint_engines=(mybir.EngineType.PE,)
            ) as iv:
                for u in range(SCAN_UNROLL):
                    t = (T - 1) - (iv + u) if rev else iv + u
                    scan_step(whh_sb, xg_dram, y_dram, t)
            if save is not None:
                nc.vector.tensor_copy(save[0][:], hT[:])
                nc.vector.tensor_copy(save[1][:], cT[:])
            barrier()

        # ---- phase 1: layer-0 input projections (table gathers) ----
        tabf_sb = wpool.tile([P, G], F16, tag="tab")
        nc.sync.dma_start(tabf_sb[:], tab_f[:])
        tabb_sb = wpool.tile([P, G], F16, tag="tab2")
        nc.sync.dma_start(tabb_sb[:], tab_b[:])
        onehot_gemm(tabf_sb, xg["af"])
        onehot_gemm(tabb_sb, xg["ab"])
        barrier()

        # ---- layer-0 scans ----
        scan(whh["l0f"], xg["af"], ybuf["l0f"], False, None, (sav_hf, sav_cf))
        scan(whh["l0b"], xg["ab"], ybuf["l0b"], True, None, (sav_hb, sav_cb))

        # ---- layer-1 input projections ----
        wf_sb = wpool.tile([P, 8 * G], F16, tag="wih")
        nc.sync.dma_start(wf_sb[:], wih_l1f[:])
        wb_sb = wpool.tile([P, 8 * G], F16, tag="wih2")
        nc.sync.dma_start(wb_sb[:], wih_l1b[:])

        def l1_rhs(nt):
            return [load_y_rhs(ybuf["l0f"], nt, k) for k in range(NK)] + [
                load_y_rhs(ybuf["l0b"], nt, k) for k in range(NK)
            ]

        for nt in range(NTILES):
            tiles = l1_rhs(nt)
            for wsb, bt, dst in ((wf_sb, bias_sb["l1f"], "bf"), (wb_sb, bias_sb["l1b"], "bb")):
                for m in range(NM):
                    ps = pspool.tile([P, NTT], F32)
                    for k in range(8):
                        nc.tensor.matmul(
                            ps[:],
                            wsb[:, k * G + m * P : k * G + (m + 1) * P],
                            tiles[k][:],
                            start=(k == 0),
                            stop=(k == 7),
                        )
                    xsb = xsbpool.tile([P, NTT], F32)
                    nc.scalar.activation(xsb[:], ps[:], AF.Identity, bias=bt[:, m : m + 1])
                    d = xg[dst][:, ds(nt * TT, TT), m * BL : (m + 1) * BL]
                    nc.sync.dma_start(d, xsb[:].rearrange("p (t b) -> p t b", b=BL))
        barrier()

        # ---- layer-1 scans ----
        scan(whh["l1f"], xg["bf"], ybuf["l1f"], False, None, None)
        scan(whh["l1b"], xg["bb"], ybuf["l1b"], True, None, None)

        # ---- decoder layer-0 input projection (emb table + enc_out GEMM) ----
        wd_sb = wpool.tile([P, 8 * G], F16, tag="wih")
        nc.sync.dma_start(wd_sb[:], wih_d0e[:])
        tabd_sb = wpool.tile([P, G], F16, tag="tab")
        nc.sync.dma_start(tabd_sb[:], tab_d[:])

        def d0_rhs(nt):
            return [load_y_rhs(ybuf["l1f"], nt, k) for k in range(NK)] + [
                load_y_rhs(ybuf["l1b"], nt, k) for k in range(NK)
            ]

        proj_gemm(wd_sb, 8, d0_rhs, None, xg["af"], extra_first=tabd_sb)
        barrier()

        # ---- decoder scans ----
        scan(whh["d0"], xg["af"], ybuf["d0"], False, (sav_hf, sav_cf), None)

        wd1_sb = wpool.tile([P, 4 * G], F16, tag="wih")
        nc.sync.dma_start(wd1_sb[:], wih_d1[:])

        def d1_rhs(nt):
            return [load_y_rhs(ybuf["d0"], nt, k) for k in range(NK)]

        proj_gemm(wd1_sb, NK, d1_rhs, bias_sb["d1"], xg["bf"])
        barrier()

        scan(whh["d1"], xg["bf"], ybuf["d1"], False, (sav_hb, sav_cb), None)

        # ---- output projection ----
        ow_sb = wpool.tile([P, NK * V], F16, tag="tab")
        nc.sync.dma_start(ow_sb[:], owT[:])
        for nt in range(NTILES):
            tiles = [load_y_rhs(ybuf["d1"], nt, k) for k in range(NK)]
            ps = pspool.tile([P, NTT], F32)
            for k in range(NK):
                nc.tensor.matmul(
                    ps[:],
                    ow_sb[:, k * V : (k + 1) * V],
                    tiles[k][:],
                    start=(k == 0),
                    stop=(k == NK - 1),
                )
            xsb = xsbpool.tile([P, NTT], F32)
            nc.scalar.activation(xsb[:], ps[:], AF.Identity, bias=bout_sb[:])
            nc.sync.dma_start(logitsT[:, nt * NTT : (nt + 1) * NTT], xsb[:])

    nc.finalize()
    return nc


# ---------------- host-side packing ----------------


def _pack_whhT(Whh):
    out = np.empty((P, NK * G), np.float16)
    for k in range(NK):
        out[:, k * G : (k + 1) * G] = Whh[:, k * P : (k + 1) * P].T
    return out


def _pack_wihT(Wih, col_off, nkc):
    out = np.empty((P, nkc * G), np.float16)
    for k in range(nkc):
        c = col_off + k * P
        out[:, k * G : (k + 1) * G] = Wih[:, c : c + P].T
    return out


def _pack_table(emb, Wih_sub, bias):
    tab = emb.astype(np.float64) @ Wih_sub.astype(np.float64).T + bias.astype(np.float64)
    return tab.astype(np.float16)  # [V, G]


def _pack_bias(bih, bhh):
    b = (bih + bhh).astype(np.float32)
    return b.reshape(NM, P).T.copy()  # [p, m]


_CACHE = {}
LAST_EXEC_NS = None
LAST_RAW_NS = None


def _run_spmd_timed(nc, in_maps, iters=3):
    """Mirror run_bass_via_pjrt's multi-core path, but device_put inputs once
    so repeated executions time (exec + dispatch), not input upload."""
    import time as _time

    import jax
    import jax.numpy as jnp
    import concourse.mybir as mybir_
    from concourse import bass2jax
    from jax.experimental.shard_map import shard_map
    from jax.sharding import Mesh, NamedSharding, PartitionSpec

    bass2jax.install_neuronx_cc_hook()
    n_cores = len(in_maps)
    partition_name = nc.partition_id_tensor.name if nc.partition_id_tensor else None

    in_names, out_names, out_avals, zero_outs = [], [], [], []
    for alloc in nc.m.functions[0].allocations:
        if not isinstance(alloc, mybir_.MemoryLocationSet):
            continue
        name = alloc.memorylocations[0].name
        if alloc.kind == "ExternalInput":
            if name != partition_name:
                in_names.append(name)
        elif alloc.kind == "ExternalOutput":
            out_names.append(name)
            shape = tuple(alloc.tensor_shape)
            dtype = mybir_.dt.np(alloc.dtype)
            out_avals.append(jax.core.ShapedArray(shape, dtype))
            zero_outs.append(np.zeros(shape, dtype))
    n_params = len(in_names)
    n_outs = len(out_avals)
    all_in_names = list(in_names) + list(out_names)
    if partition_name is not None:
        all_in_names.append(partition_name)

    donate = tuple(range(n_params, n_params + n_outs))

    def _body(*args):
        operands = list(args)
        if partition_name is not None:
            operands.append(bass2jax.partition_id_tensor())
        outs = bass2jax._bass_exec_p.bind(
            *operands,
            out_avals=tuple(out_avals),
            in_names=tuple(all_in_names),
            out_names=tuple(out_names),
            lowering_input_output_aliases=(),
            sim_require_finite=True,
            sim_require_nnan=True,
            nc=nc,
        )
        return tuple(outs)

    devices = jax.devices()[:n_cores]
    mesh = Mesh(np.asarray(devices), ("core",))
    in_specs = (PartitionSpec("core"),) * (n_params + n_outs)
    out_specs = (PartitionSpec("core"),) * len(out_names)
    sharded = jax.jit(
        shard_map(_body, mesh=mesh, in_specs=in_specs, out_specs=out_specs, check_rep=False),
        donate_argnums=donate,
        keep_unused=True,
    )
    shd = NamedSharding(mesh, PartitionSpec("core"))
    concat_in = [
        jax.device_put(
            np.concatenate([np.asarray(in_maps[c][nm]) for c in range(n_cores)], axis=0),
            shd,
        )
        for nm in in_names
    ]
    big_zeros = [np.concatenate([z] * n_cores, axis=0) for z in zero_outs]

    best = None
    out_arrs = None
    for _ in range(max(1, iters)):
        zo = [jax.device_put(z, shd) for z in big_zeros]
        jax.block_until_ready(zo)
        jax.block_until_ready(concat_in)
        t0 = _time.perf_counter()
        out_arrs = sharded(*concat_in, *zo)
        jax.block_until_ready(out_arrs)
        dt = _time.perf_counter() - t0
        best = dt if best is None else min(best, dt)

    results = []
    for c in range(n_cores):
        d = {}
        for i, nm in enumerate(out_names):
            full = np.asarray(out_arrs[i])
            per = full.shape[0] // n_cores
            d[nm] = full[c * per : (c + 1) * per]
        results.append(d)
    return results, best


def _build_tiny():
    """Trivial kernel used to calibrate per-dispatch overhead."""
    nc = bacc.Bacc(None, target_bir_lowering=False)
    a = nc.dram_tensor("a", [P, P], F32, kind="ExternalInput")
    o = nc.dram_tensor("o", [P, P], F32, kind="ExternalOutput")
    with tile.TileContext(nc) as tc, ExitStack() as ctx:
        pool = ctx.enter_context(tc.tile_pool(name="p", bufs=1))
        t = pool.tile([P, P], F32)
        nc.sync.dma_start(t[:], a[:])
        nc.sync.dma_start(o[:], t[:])
    nc.finalize()
    return nc


def dispatch_baseline_ns(iters=5):
    nc = _CACHE.get("tiny")
    if nc is None:
        nc = _CACHE["tiny"] = _build_tiny()
    a = np.zeros((P, P), np.float32)
    _, best = _run_spmd_timed(nc, [{"a": a}] * NCORES, iters=iters)
    return int(best * 1e9)


def kernel(**inp):
    x = np.asarray(inp["x"])
    B_, T = x.shape
    assert B_ == B
    TOK = BL * T

    key = T
    if key not in _CACHE:
        nc = bacc.Bacc(None, target_bir_lowering=False)
        build_model(nc, T)
        _CACHE[key] = nc
    nc = _CACHE[key]

    emb = inp["emb"].astype(np.float32)
    common = {
        "tab_f": _pack_table(emb, inp["enc_Wih_l0f"], inp["enc_bih_l0f"] + inp["enc_bhh_l0f"]),
        "tab_b": _pack_table(emb, inp["enc_Wih_l0b"], inp["enc_bih_l0b"] + inp["enc_bhh_l0b"]),
        "tab_d": _pack_table(
            emb, inp["dec_Wih_l0"][:, :E], inp["dec_bih_l0"] + inp["dec_bhh_l0"]
        ),
        "whhT_l0f": _pack_whhT(inp["enc_Whh_l0f"]),
        "whhT_l0b": _pack_whhT(inp["enc_Whh_l0b"]),
        "whhT_l1f": _pack_whhT(inp["enc_Whh_l1f"]),
        "whhT_l1b": _pack_whhT(inp["enc_Whh_l1b"]),
        "whhT_d0": _pack_whhT(inp["dec_Whh_l0"]),
        "whhT_d1": _pack_whhT(inp["dec_Whh_l1"]),
        "wihT_l1f": _pack_wihT(inp["enc_Wih_l1f"], 0, 8),
        "wihT_l1b": _pack_wihT(inp["enc_Wih_l1b"], 0, 8),
        "wihT_d0e": _pack_wihT(inp["dec_Wih_l0"], E, 8),
        "wihT_d1": _pack_wihT(inp["dec_Wih_l1"], 0, NK),
        "owT": np.concatenate(
            [inp["out_W"][:, k * P : (k + 1) * P].T for k in range(NK)], axis=1
        ).astype(np.float16),
        "bias_l1f": _pack_bias(inp["enc_bih_l1f"], inp["enc_bhh_l1f"]),
        "bias_l1b": _pack_bias(inp["enc_bih_l1b"], inp["enc_bhh_l1b"]),
        "bias_d1": _pack_bias(inp["dec_bih_l1"], inp["dec_bhh_l1"]),
        "bias_out": inp["out_b"].astype(np.float32).reshape(P, 1),
    }

    in_maps = []
    for c in range(NCORES):
        xl = x[c * BL : (c + 1) * BL].astype(np.int64)  # [BL, T]
        oh = np.zeros((V, TOK), np.float16)
        oh[xl.T.reshape(-1), np.arange(TOK)] = 1.0  # col j = t*BL+b
        in_maps.append({**common, "onehotT": oh})

    results, best_s = _run_spmd_timed(nc, in_maps, iters=3)
    global LAST_EXEC_NS
    LAST_EXEC_NS = int(best_s * 1e9)
    global LAST_RAW_NS
    LAST_RAW_NS = int(best_s * 1e9)

    out = np.empty((B, T, V), np.float32)
    for c in range(NCORES):
        lt = results[c]["logitsT"]  # [V, TOK]
        out[c * BL : (c + 1) * BL] = lt.reshape(V, T, BL).transpose(2, 1, 0)
    return out
